# revision 6
# baseline (speedup 1.0000x reference)
"""AttnBlock (GroupNorm -> qkv 1x1 conv -> 8-head attention over 32x32
spatial -> proj 1x1 conv -> residual) on 8 Trainium2 NeuronCores.

Sharding: fully data-parallel, no collectives. Core i handles batch
b = i//2 and query-half s = i%2 (512 of the 1024 spatial positions).

v2 restructure (from trace analysis of the 106us baseline):
  - PE column bus is the tensor budget (~0.42 ns/col); total ~115k
    512-col streams ~= 48us.  Scalar engine only does softmax exps
    (32 x [128,1024] ~= 33us) - all bias/identity work moved to DVE.
  - Input DMA posted as a few big linear descriptors spread over 5
    engine queues; host pre-arranges every tensor into its exact SBUF
    tile layout so every transfer is fully contiguous.
  - Attention for head-pair 0 starts as soon as k0/q0 exist; the
    remaining k/q/v projection matmuls ride as filler between
    exp-paced score/AV streams.
  - 1/Z via DVE reciprocal on the PSUM Z-rows directly,
    broadcast with two tiny K=1 expander matmuls per pair (no Ln/Exp,
    no rz staging DMAs).
  - proj accumulated in SBUF f32 (psum banks stay free for the
    attention pipeline); residual+bias folded into one tail add per
    m-tile; bf16 output DMA (host upcasts).

Toolchain workarounds: the Tile-tail Drain and any instruction carrying
more than one semaphore wait are rejected by this walrus build, so
excess waits are spread onto same-engine NoOps post-schedule.
"""

import os

import numpy as np

import concourse.bass as bass
import concourse.tile as tile
from concourse import mybir
from concourse.bass_utils import run_bass_kernel_spmd
from concourse.vector_clock import ScopedClock

# ---------------------------------------------------------------------------
# walrus workaround: the Tile kernel-tail Drain may carry more sem waits than
# the CTRL instruction encoding allows; spread them over sync-engine NOPs.
_MAX_WAITS_PER_INST = 1


def _patched_drain_and_barrier(self, tick_clock, wait_clock):
    nc = self.nc
    probe = nc.sync.nop(nofuse=True, hint="drain_wait_spread")
    wait_clock.add_sem_waits(probe.ins, ScopedClock({None: tick_clock.global_clock}))
    si = probe.ins.sync_info
    waits = list(si.on_wait) if si is not None else []
    if len(waits) > _MAX_WAITS_PER_INST:
        probe.ins.sync_info = mybir.SyncInfo(
            on_wait=waits[:_MAX_WAITS_PER_INST], on_update=[]
        )
        for i in range(_MAX_WAITS_PER_INST, len(waits), _MAX_WAITS_PER_INST):
            nop = nc.sync.nop(nofuse=True, hint="drain_wait_spread")
            nop.ins.sync_info = mybir.SyncInfo(
                on_wait=waits[i : i + _MAX_WAITS_PER_INST], on_update=[]
            )
    nc.sync.drain()
    nc.all_engine_barrier(sem_only=True)
    popped = nc._tile_sem_poison_stack.pop()
    assert popped is self._sem_poison
    nc.clear_and_free_semaphores(list(self.sems.allocated().values()))


tile.TileContext._drain_and_barrier = _patched_drain_and_barrier


def _split_multi_waits(nc, max_waits=1):
    """walrus rejects instructions with more than one sem wait; move the
    excess onto same-engine NoOps placed immediately before."""
    ctr = 0
    for blk in nc.m.functions[0].blocks:
        out = []
        for inst in blk.instructions:
            si = inst.sync_info
            waits = list(si.on_wait) if (si and si.on_wait) else []
            if len(waits) > max_waits:
                extra, keep = waits[:-max_waits], waits[-max_waits:]
                for j in range(0, len(extra), max_waits):
                    ctr += 1
                    nop = mybir.InstNoOp(name=f"I-wsplit-{ctr}")
                    nop.engine = inst.engine
                    nop.sync_info = mybir.SyncInfo(
                        on_wait=extra[j : j + max_waits], on_update=[])
                    out.append(nop)
                inst.sync_info = mybir.SyncInfo(
                    on_wait=keep,
                    on_update=list(si.on_update) if si.on_update else [])
            out.append(inst)
        blk.instructions = out
    return ctr
# ---------------------------------------------------------------------------

B = 4
C = 512
H = W = 32
HWF = 1024  # keys / full spatial
Q = 512  # queries per core (half of HWF)
NH = 8
CHD = 64  # channels per head
CT = 4  # 128-channel tiles of C
KT = 8  # 128-key tiles of HWF
GROUPS = 32
GPC = 16  # channels per group
EPS = 1e-6
F32 = mybir.dt.float32
BF16 = mybir.dt.bfloat16
DT = BF16
_DT_NAME = "bf16"


def build_program():
    nc = bass.Bass("TRN2", target_bir_lowering=False, debug=False, num_devices=8)

    def din(name, shape, dt=BF16):
        return nc.declare_dram_parameter(name, list(shape), dt, isOutput=False)

    kv_d = din("kv", [128, CT * HWF])      # [p, t*1024 + j]
    xs_d = din("xs", [128, CT * Q])        # [p, t*512 + q]  (this core's half)
    xo_d = din("xo", [128, CT * Q])        # other half (stats only)
    wk_d = din("wk", [128, CT * C])        # [p=in-chan of k-tile, k*512 + o]
    wq_d = din("wq", [128, CT * C])
    wv_d = din("wv", [128, CT * C])
    wp_d = din("wp", [128, CT * C])
    cpack_d = din("cpack", [128, 36], F32)
    e16_d = din("e16", [8, 128], F32)
    bv_d = din("bv", [C], F32)
    out_d = nc.declare_dram_parameter("out", [128, CT * Q], BF16, isOutput=True)

    from contextlib import ExitStack
    with tile.TileContext(nc) as tc, ExitStack() as ctx:
        cst = ctx.enter_context(tc.tile_pool(name="cst", bufs=1))
        big = ctx.enter_context(tc.tile_pool(name="big", bufs=1))
        wrk = ctx.enter_context(tc.tile_pool(name="wrk", bufs=3))
        epool = ctx.enter_context(tc.tile_pool(name="epool", bufs=4))
        ps_s = ctx.enter_context(tc.tile_pool(name="ps_s", bufs=2, space="PSUM"))
        ps_o = ctx.enter_context(tc.tile_pool(name="ps_o", bufs=1, space="PSUM"))
        ps_mm = ctx.enter_context(tc.tile_pool(name="ps_mm", bufs=2, space="PSUM"))

        # ---- input DMA: few big linear descriptors over 5 queues ----
        cpk = cst.tile([128, 36], F32)
        nc.scalar.dma_start(cpk[:], cpack_d[:])
        e16 = cst.tile([8, 128], F32)
        nc.scalar.dma_start(e16[:], e16_d[:])
        bq_c, bk_c, bp_c = cpk[:, 0:4], cpk[:, 4:8], cpk[:, 8:12]
        gqs_c, gqb_c = cpk[:, 12:16], cpk[:, 16:20]
        gks_c, gkb_c = cpk[:, 20:24], cpk[:, 24:28]
        g16 = cpk[:, 28:36]

        kvt = []
        for t in range(CT):
            kt_ = big.tile([128, HWF], BF16, name=f"kv{t}")
            nc.gpsimd.dma_start(kt_[:], kv_d[:, t * HWF : (t + 1) * HWF])
            kvt.append(kt_)
        xs = big.tile([128, CT * Q], BF16, name="xs")
        nc.sync.dma_start(xs[:], xs_d[:])
        xo = big.tile([128, CT * Q], BF16, name="xo")
        nc.gpsimd.dma_start(xo[:], xo_d[:])
        wk_sb = big.tile([128, CT * C], BF16, name="wk")
        nc.scalar.dma_start(wk_sb[:], wk_d[:])
        wq_sb = big.tile([128, CT * C], BF16, name="wq")
        nc.scalar.dma_start(wq_sb[:], wq_d[:])
        wv_sb = big.tile([128, CT * C], BF16, name="wv")
        nc.sync.dma_start(wv_sb[:], wv_d[:])
        wp_sb = big.tile([128, CT * C], BF16, name="wp")
        nc.sync.dma_start(wp_sb[:], wp_d[:])
        bv_ap = bv_d[:]
        bvbc = cst.tile([128, C], F32)
        nc.gpsimd.dma_start(
            out=bvbc[:],
            in_=bass.AP(tensor=bv_ap.tensor, offset=bv_ap.offset,
                        ap=[[0, 128]] + list(bv_ap.ap)),
        )

        # ---- warmup: preload the exp/ln ACT table set at t~0 ----
        wtin = wrk.tile([1, 4], F32, name="wtin", bufs=1)
        nc.vector.memset(wtin[:], 0.0)
        wtout = wrk.tile([1, 4], F32, name="wtout", bufs=1)
        nc.scalar.activation(wtout[:], wtin[:],
                             mybir.ActivationFunctionType.Exp)

        # ---- groupnorm affine coefficients (a, b per channel) ----
        def gn_coeffs(src_chunks, gam, bet, label):
            statc = wrk.tile([128, 8], F32, name=f"statc_{label}", bufs=1)
            for t in range(CT):
                chunks = src_chunks[t]
                bnst = wrk.tile([128, len(chunks), 6], F32,
                                name=f"bnst_{label}", tag="bnst")
                for half, chunk in enumerate(chunks):
                    nc.vector.bn_stats(out=bnst[:, half, :], in_=chunk)
                mv = wrk.tile([128, 2], F32, name=f"mv_{label}", tag="mv")
                nc.vector.bn_aggr(out=mv[:], in_=bnst[:])
                nc.vector.tensor_copy(statc[:, t : t + 1], mv[:, 0:1])
                msq = wrk.tile([128, 1], F32, name=f"msq_{label}", tag="msq")
                nc.vector.tensor_mul(msq[:], mv[:, 0:1], mv[:, 0:1])
                nc.vector.tensor_add(statc[:, 4 + t : 5 + t], msq[:], mv[:, 1:2])
            gps = ps_mm.tile([128, 512], F32, name=f"gps_{label}", tag="mm")
            nc.tensor.matmul(gps[0:8, 0:8], lhsT=g16, rhs=statc[:],
                             start=True, stop=True)
            gs = wrk.tile([8, 8], F32, name=f"gs_{label}", tag="gs")
            nc.vector.tensor_copy(gs[:], gps[0:8, 0:8])
            ms = wrk.tile([8, 8], F32, name=f"ms_{label}", tag="ms")
            nc.vector.tensor_scalar_mul(ms[:], gs[:], 1.0 / GPC)
            msq8 = wrk.tile([8, 4], F32, name=f"msq8_{label}", tag="msq8")
            nc.vector.tensor_mul(msq8[:], ms[:, 0:4], ms[:, 0:4])
            var8 = wrk.tile([8, 4], F32, name=f"var8_{label}", tag="var8")
            nc.vector.tensor_sub(var8[:], ms[:, 4:8], msq8[:])
            # rstd = exp(-0.5*ln(var+eps)) — Ln/Exp share one ACT table set
            lnv = wrk.tile([8, 4], F32, name=f"lnv_{label}", tag="lnv")
            eps8 = wrk.tile([8, 1], F32, name=f"eps8_{label}", tag="eps8")
            nc.vector.memset(eps8[:], EPS)
            nc.scalar.activation(lnv[:], var8[:],
                                 mybir.ActivationFunctionType.Ln, bias=eps8[:])
            rhs2 = wrk.tile([8, 8], F32, name=f"rhs2_{label}", tag="rhs2", bufs=1)
            nc.scalar.activation(rhs2[:, 0:4], lnv[:],
                                 mybir.ActivationFunctionType.Exp, scale=-0.5)
            nc.vector.tensor_copy(rhs2[:, 4:8], ms[:, 0:4])
            pcs = ps_mm.tile([128, 512], F32, name=f"pcs_{label}", tag="mm")
            nc.tensor.matmul(pcs[:, 0:8], lhsT=e16[:], rhs=rhs2[:],
                             start=True, stop=True)
            pc = wrk.tile([128, 8], F32, name=f"pc_{label}", tag="pc")
            nc.vector.tensor_copy(pc[:], pcs[:, 0:8])
            a = wrk.tile([128, 4], F32, name=f"a_{label}", bufs=1)
            nc.vector.tensor_mul(a[:], pc[:, 0:4], gam)
            tmpb = wrk.tile([128, 4], F32, name=f"tmpb_{label}", tag="tmpb")
            nc.vector.tensor_mul(tmpb[:], pc[:, 4:8], a[:])
            b = wrk.tile([128, 4], F32, name=f"b_{label}", bufs=1)
            nc.vector.tensor_sub(b[:], bet, tmpb[:])
            return a, b

        akv, bkv = gn_coeffs(
            [(kvt[t][:, 0:512], kvt[t][:, 512:1024]) for t in range(CT)],
            gks_c, gkb_c, "kv")

        kvn = []
        for t in range(CT):
            kh = big.tile([128, HWF], DT, name=f"kvn{t}")
            nc.vector.tensor_scalar(
                out=kh[:], in0=kvt[t][:],
                scalar1=akv[:, t : t + 1], scalar2=bkv[:, t : t + 1],
                op0=mybir.AluOpType.mult, op1=mybir.AluOpType.add)
            kvn.append(kh)

        ax, bx = gn_coeffs(
            [(xs[:, t * Q : t * Q + 512], xo[:, t * Q : t * Q + 512])
             for t in range(CT)],
            gqs_c, gqb_c, "x")
        qin = []
        for t in range(CT):
            qt = big.tile([128, Q], DT, name=f"qin{t}")
            nc.vector.tensor_scalar(
                out=qt[:], in0=xs[:, t * Q : (t + 1) * Q],
                scalar1=ax[:, t : t + 1], scalar2=bx[:, t : t + 1],
                op0=mybir.AluOpType.mult, op1=mybir.AluOpType.add)
            qin.append(qt)

        k_sb = [None] * CT
        q_sb = [None] * CT
        vT_sb = [None] * KT

        def emit_k(m):
            kt_ = big.tile([128, HWF], DT, name=f"k{m}")
            for nh in range(2):
                ps = ps_mm.tile([128, 512], F32, name=f"psk{m}{nh}", tag="mm")
                for k in range(CT):
                    nc.tensor.matmul(
                        ps[:],
                        lhsT=wk_sb[:, k * C + m * 128 : k * C + (m + 1) * 128],
                        rhs=kvn[k][:, nh * 512 : (nh + 1) * 512],
                        start=(k == 0), stop=(k == CT - 1))
                nc.vector.tensor_scalar_add(
                    kt_[:, nh * 512 : (nh + 1) * 512], ps[:],
                    bk_c[:, m : m + 1])
            k_sb[m] = kt_

        def emit_q(m):
            ps = ps_mm.tile([128, 512], F32, name=f"psq{m}", tag="mm")
            for k in range(CT):
                nc.tensor.matmul(
                    ps[:],
                    lhsT=wq_sb[:, k * C + m * 128 : k * C + (m + 1) * 128],
                    rhs=qin[k][:], start=(k == 0), stop=(k == CT - 1))
            qt = big.tile([128, Q], DT, name=f"q{m}")
            nc.vector.tensor_scalar_add(qt[:], ps[:], bq_c[:, m : m + 1])
            q_sb[m] = qt

        def emit_v(mt):
            vt = big.tile([128, NH * (CHD + 1)], DT, name=f"vT{mt}")
            ones_col = vt[:].rearrange("p (h c) -> p h c", c=CHD + 1)[
                :, :, CHD : CHD + 1]
            nc.vector.memset(ones_col, 1.0)
            ps = ps_mm.tile([128, 512], F32, name=f"psv{mt}", tag="mm")
            for k in range(CT):
                nc.tensor.matmul(
                    ps[:], lhsT=kvn[k][:, mt * 128 : (mt + 1) * 128],
                    rhs=wv_sb[:, k * C : (k + 1) * C],
                    start=(k == 0), stop=(k == CT - 1))
            nc.vector.tensor_tensor(
                out=vt[:].rearrange("p (h c) -> p h c", c=CHD + 1)[:, :, 0:CHD],
                in0=ps[:].rearrange("p (h c) -> p h c", c=CHD),
                in1=bvbc[:].rearrange("p (h c) -> p h c", c=CHD),
                op=mybir.AluOpType.add)
            vT_sb[mt] = vt

        # ---- attention machinery (head pairs t) ----
        # zexp row-64 ones for the two K=1 broadcast expanders per pair
        zexp = cst.tile([65, 128], DT, name="zexp")
        nc.vector.memset(zexp[64:65, :], 1.0)

        et_tiles = {}

        def emit_scores(t, mk):
            pss = ps_s.tile([128, 1024], F32, name=f"pss{t}{mk}", tag="s")
            nc.tensor.matmul(pss[:, 0:512],
                             lhsT=k_sb[t][0:64, mk * 128 : (mk + 1) * 128],
                             rhs=q_sb[t][0:64, :],
                             start=True, stop=True, tile_position=(0, 0))
            nc.tensor.matmul(pss[:, 512:1024],
                             lhsT=k_sb[t][64:128, mk * 128 : (mk + 1) * 128],
                             rhs=q_sb[t][64:128, :],
                             start=True, stop=True, tile_position=(64, 0))
            et = epool.tile([128, 1024], DT, name=f"e{t}{mk}", tag="e")
            nc.scalar.activation(et[:], pss[:],
                                 mybir.ActivationFunctionType.Exp,
                                 scale=float(CHD) ** -0.5)
            et_tiles[(t, mk)] = et

        po_pair = {}

        def emit_av(t, mk):
            if mk == 0:
                po_pair[t] = (
                    ps_o.tile([65, 512], F32, name=f"poA{t}", tag="oA"),
                    ps_o.tile([65, 512], F32, name=f"poB{t}", tag="oB"),
                )
            poA, poB = po_pair[t]
            et = et_tiles[(t, mk)]
            nc.tensor.matmul(poA[:],
                             lhsT=vT_sb[mk][:, 130 * t : 130 * t + 65],
                             rhs=et[:, 0:512],
                             start=(mk == 0), stop=(mk == KT - 1))
            nc.tensor.matmul(poB[:],
                             lhsT=vT_sb[mk][:, 130 * t + 65 : 130 * t + 130],
                             rhs=et[:, 512:1024],
                             start=(mk == 0), stop=(mk == KT - 1))

        on_sb = [None] * CT

        def finish_pair(t):
            """AV(t,7) done: stage outputs, 1/Z, broadcast, normalize."""
            poA, poB = po_pair[t]
            ost = wrk.tile([128, 512], F32, name=f"ost{t}", tag="ost", bufs=2)
            nc.vector.tensor_copy(ost[0:64, :], poA[0:64, :])
            stB = wrk.tile([65, 512], F32, name=f"stB{t}", tag="stB", bufs=2)
            nc.vector.tensor_copy(stB[0:65, :], poB[0:65, :])
            # 1/Z straight off the PSUM Z-rows (row 64 = ones-col matmul)
            zrec = wrk.tile([65, 1024], F32, name=f"zrec{t}", tag="zrec", bufs=2)
            nc.vector.reciprocal(zrec[64:65, 0:512], poA[64:65, :])
            nc.vector.reciprocal(zrec[64:65, 512:1024], poB[64:65, :])
            rzb = wrk.tile([65, 1024], DT, name=f"rzb{t}", tag="rzb", bufs=2)
            nc.vector.tensor_copy(rzb[64:65, :], zrec[64:65, :])
            # B-half data under A (partition shift via DMA)
            nc.sync.dma_start(ost[64:128, :], stB[0:64, :])
            # zps[0:64] = 1/Z_A broadcast, zps[64:128] = 1/Z_B broadcast
            zps = ps_mm.tile([128, 512], F32, name=f"zps{t}", tag="mm")
            nc.tensor.matmul(zps[0:64, :], lhsT=zexp[64:65, 0:64],
                             rhs=rzb[64:65, 0:512], start=True, stop=True,
                             tile_position=(64, 0))
            nc.tensor.matmul(zps[64:128, :], lhsT=zexp[64:65, 64:128],
                             rhs=rzb[64:65, 512:1024], start=True, stop=True,
                             tile_position=(64, 64))
            ont = big.tile([128, Q], DT, name=f"on{t}")
            nc.vector.tensor_mul(ont[:], ost[:], zps[:])
            on_sb[t] = ont

        # proj: SBUF f32 accumulation so PSUM stays free for attention
        acc = [big.tile([128, Q], F32, name=f"acc{m}") for m in range(CT)]
        xbp = [big.tile([128, Q], F32, name=f"xbp{m}") for m in range(CT)]

        def emit_xbp(m):
            nc.gpsimd.tensor_scalar_add(
                xbp[m][:], xs[:, m * Q : (m + 1) * Q], bp_c[:, m : m + 1])

        def emit_proj(m, k):
            ps = ps_mm.tile([128, 512], F32, name=f"psp{m}{k}", tag="mm")
            nc.tensor.matmul(
                ps[:], lhsT=wp_sb[:, k * C + m * 128 : k * C + (m + 1) * 128],
                rhs=on_sb[k][:], start=True, stop=True)
            if k == 0:
                nc.vector.tensor_tensor(out=acc[m][:], in0=ps[:], in1=xbp[m][:],
                                        op=mybir.AluOpType.add)
            elif k < CT - 1:
                nc.vector.tensor_tensor(out=acc[m][:], in0=ps[:], in1=acc[m][:],
                                        op=mybir.AluOpType.add)
            else:
                r2 = wrk.tile([128, Q], BF16, name=f"r2_{m}", tag="r2", bufs=2)
                nc.vector.tensor_tensor(out=r2[:], in0=ps[:], in1=acc[m][:],
                                        op=mybir.AluOpType.add)
                eng = nc.sync if m % 2 == 0 else nc.gpsimd
                eng.dma_start(out_d[:, m * Q : (m + 1) * Q], r2[:])

        # ------------------ tensor-engine program order ------------------
        emit_k(0)
        emit_q(0)
        for m in range(CT):
            emit_xbp(m)
        emit_scores(0, 0)
        emit_scores(0, 1)
        emit_v(0)
        emit_v(1)
        # t=0: AV paced by exp; v tiles + k1/q1 as filler
        for mk in range(KT):
            emit_av(0, mk)
            if mk + 2 < KT:
                emit_scores(0, mk + 2)
            if mk + 2 < KT:
                emit_v(mk + 2)
        emit_k(1)
        emit_q(1)
        finish_pair(0)
        emit_scores(1, 0)
        emit_scores(1, 1)
        # t=1..3 with filler: k2/q2 during t=1, k3/q3 during t=2,
        # proj rounds as on(t) becomes available during t>=1
        filler = {1: [lambda: emit_k(2), lambda: emit_q(2),
                      lambda: emit_proj(0, 0), lambda: emit_proj(1, 0),
                      lambda: emit_proj(2, 0), lambda: emit_proj(3, 0)],
                  2: [lambda: emit_k(3), lambda: emit_q(3),
                      lambda: emit_proj(0, 1), lambda: emit_proj(1, 1),
                      lambda: emit_proj(2, 1), lambda: emit_proj(3, 1)],
                  3: [lambda: emit_proj(0, 2), lambda: emit_proj(1, 2),
                      lambda: emit_proj(2, 2), lambda: emit_proj(3, 2)]}
        for t in range(1, CT):
            fl = list(filler[t])
            for mk in range(KT):
                emit_av(t, mk)
                if mk + 2 < KT:
                    emit_scores(t, mk + 2)
                if fl:
                    fl.pop(0)()
            for f in fl:
                f()
            finish_pair(t)
            if t + 1 < CT:
                emit_scores(t + 1, 0)
                emit_scores(t + 1, 1)
        for m in range(CT):
            emit_proj(m, 3)

    _split_multi_waits(nc)
    return nc


_NC_CACHE = None
LAST_EXEC_NS = None


def kernel(**inputs):
    global _NC_CACHE, LAST_EXEC_NS
    import ml_dtypes
    bf = ml_dtypes.bfloat16

    x = np.asarray(inputs["x"], dtype=np.float32)
    kv = np.asarray(inputs["kv"], dtype=np.float32)

    def wtile(w):
        # [out,in] -> lhsT tiles: [128 p=in-chan within k-tile, k*512 + out]
        wT = np.asarray(w, np.float32).T  # [in, out]
        t = wT.reshape(CT, 128, C).transpose(1, 0, 2).reshape(128, CT * C)
        return np.ascontiguousarray(t).astype(bf)

    wkT = wtile(inputs["wk"])
    wqT = wtile(inputs["wq"])
    wvT = wtile(inputs["wv"])
    wpT = wtile(inputs["wproj"])
    bq = np.asarray(inputs["bq"], np.float32)
    bk = np.asarray(inputs["bk"], np.float32)
    bv = np.asarray(inputs["bv"], np.float32)
    bp = np.asarray(inputs["bproj"], np.float32)
    gqs = np.asarray(inputs["gnq_scale"], np.float32)
    gqb = np.asarray(inputs["gnq_bias"], np.float32)
    gks = np.asarray(inputs["gnkv_scale"], np.float32)
    gkb = np.asarray(inputs["gnkv_bias"], np.float32)

    p = np.arange(128)
    g16 = (p[:, None] // GPC == np.arange(8)[None, :]).astype(np.float32)
    e16 = np.ascontiguousarray(g16.T)
    cpack = np.concatenate(
        [v.reshape(4, 128).T for v in (bq, bk, bp, gqs, gqb, gks, gkb)]
        + [g16], axis=1).astype(np.float32)
    cpack = np.ascontiguousarray(cpack)

    def ctile(a):  # [C, n] -> [128, CT*n] tile-major
        n = a.shape[1]
        return np.ascontiguousarray(
            a.reshape(CT, 128, n).transpose(1, 0, 2).reshape(128, CT * n))

    xr = x.reshape(B, C, HWF)
    kvr = kv.reshape(B, C, HWF)

    in_maps = []
    for core in range(8):
        b, s = core // 2, core % 2
        in_maps.append({
            "kv": ctile(kvr[b]).astype(bf),
            "xs": ctile(xr[b][:, s * Q : (s + 1) * Q]).astype(bf),
            "xo": ctile(xr[b][:, (1 - s) * Q : (2 - s) * Q]).astype(bf),
            "wk": wkT, "wq": wqT, "wv": wvT, "wp": wpT,
            "bv": bv, "cpack": cpack, "e16": e16,
        })

    if _NC_CACHE is None:
        _NC_CACHE = build_program()

    trace = os.environ.get("BASS_ATTN_TRACE", "0") == "1"
    res = run_bass_kernel_spmd(_NC_CACHE, in_maps, core_ids=list(range(8)),
                               trace=trace)
    LAST_EXEC_NS = res.exec_time_ns

    out = np.empty((B, C, HWF), np.float32)
    for core in range(8):
        b, s = core // 2, core % 2
        o = np.asarray(res.results[core]["out"]).astype(np.float32)
        o = o.reshape(128, CT, Q).transpose(1, 0, 2).reshape(C, Q)
        out[b][:, s * Q : (s + 1) * Q] = o
    return out.reshape(B, C, H, W)


# revision 7
# speedup vs baseline: 1.6773x; 1.6773x over previous
"""AttnBlock (GroupNorm -> qkv 1x1 conv -> 8-head attention over 32x32
spatial -> proj 1x1 conv -> residual) on 8 Trainium2 NeuronCores.

Sharding: fully data-parallel, no collectives. Core i handles batch
b = i//2 and query-half s = i%2 (512 of the 1024 spatial positions).

v2 restructure (from trace analysis of the 106us baseline):
  - PE column bus is the tensor budget (~0.42 ns/col); total ~115k
    512-col streams ~= 48us.  Scalar engine only does softmax exps
    (32 x [128,1024] ~= 33us) - all bias/identity work moved to DVE.
  - Input DMA posted as a few big linear descriptors spread over 5
    engine queues; host pre-arranges every tensor into its exact SBUF
    tile layout so every transfer is fully contiguous.
  - Attention for head-pair 0 starts as soon as k0/q0 exist; the
    remaining k/q/v projection matmuls ride as filler between
    exp-paced score/AV streams.
  - 1/Z = exp(-ln Z) read straight off the PSUM Z-rows, broadcast
    with two tiny K=1 expander matmuls per pair (no rz staging DMAs).
  - proj accumulated in SBUF f32 (psum banks stay free for the
    attention pipeline); residual+bias folded into one tail add per
    m-tile; bf16 output DMA (host upcasts).

Toolchain workarounds: the Tile-tail Drain and any instruction carrying
more than one semaphore wait are rejected by this walrus build, so
excess waits are spread onto same-engine NoOps post-schedule.
"""

import os

import numpy as np

import concourse.bass as bass
import concourse.tile as tile
from concourse import mybir
from concourse.bass_utils import run_bass_kernel_spmd
from concourse.vector_clock import ScopedClock

# ---------------------------------------------------------------------------
# walrus workaround: the Tile kernel-tail Drain may carry more sem waits than
# the CTRL instruction encoding allows; spread them over sync-engine NOPs.
_MAX_WAITS_PER_INST = 1


def _patched_drain_and_barrier(self, tick_clock, wait_clock):
    nc = self.nc
    probe = nc.sync.nop(nofuse=True, hint="drain_wait_spread")
    wait_clock.add_sem_waits(probe.ins, ScopedClock({None: tick_clock.global_clock}))
    si = probe.ins.sync_info
    waits = list(si.on_wait) if si is not None else []
    if len(waits) > _MAX_WAITS_PER_INST:
        probe.ins.sync_info = mybir.SyncInfo(
            on_wait=waits[:_MAX_WAITS_PER_INST], on_update=[]
        )
        for i in range(_MAX_WAITS_PER_INST, len(waits), _MAX_WAITS_PER_INST):
            nop = nc.sync.nop(nofuse=True, hint="drain_wait_spread")
            nop.ins.sync_info = mybir.SyncInfo(
                on_wait=waits[i : i + _MAX_WAITS_PER_INST], on_update=[]
            )
    nc.sync.drain()
    nc.all_engine_barrier(sem_only=True)
    popped = nc._tile_sem_poison_stack.pop()
    assert popped is self._sem_poison
    nc.clear_and_free_semaphores(list(self.sems.allocated().values()))


tile.TileContext._drain_and_barrier = _patched_drain_and_barrier


def _split_multi_waits(nc, max_waits=1):
    """walrus rejects instructions with more than one sem wait; move the
    excess onto same-engine NoOps placed immediately before."""
    ctr = 0
    for blk in nc.m.functions[0].blocks:
        out = []
        for inst in blk.instructions:
            si = inst.sync_info
            waits = list(si.on_wait) if (si and si.on_wait) else []
            if len(waits) > max_waits:
                extra, keep = waits[:-max_waits], waits[-max_waits:]
                for j in range(0, len(extra), max_waits):
                    ctr += 1
                    nop = mybir.InstNoOp(name=f"I-wsplit-{ctr}")
                    nop.engine = inst.engine
                    nop.sync_info = mybir.SyncInfo(
                        on_wait=extra[j : j + max_waits], on_update=[])
                    out.append(nop)
                inst.sync_info = mybir.SyncInfo(
                    on_wait=keep,
                    on_update=list(si.on_update) if si.on_update else [])
            out.append(inst)
        blk.instructions = out
    return ctr
# ---------------------------------------------------------------------------

B = 4
C = 512
H = W = 32
HWF = 1024  # keys / full spatial
Q = 512  # queries per core (half of HWF)
NH = 8
CHD = 64  # channels per head
CT = 4  # 128-channel tiles of C
KT = 8  # 128-key tiles of HWF
GROUPS = 32
GPC = 16  # channels per group
EPS = 1e-6
F32 = mybir.dt.float32
BF16 = mybir.dt.bfloat16
DT = BF16
_DT_NAME = "bf16"


def build_program():
    nc = bass.Bass("TRN2", target_bir_lowering=False, debug=False, num_devices=8)

    def din(name, shape, dt=BF16):
        return nc.declare_dram_parameter(name, list(shape), dt, isOutput=False)

    kv_d = din("kv", [128, CT * HWF])      # [p, t*1024 + j]
    xs_d = din("xs", [128, CT * Q])        # [p, t*512 + q]  (this core's half)
    xo_d = din("xo", [128, CT * Q])        # other half (stats only)
    wk_d = din("wk", [128, CT * C])        # [p=in-chan of k-tile, k*512 + o]
    wq_d = din("wq", [128, CT * C])
    wv_d = din("wv", [128, CT * C])
    wp_d = din("wp", [128, CT * C])
    cpack_d = din("cpack", [128, 36], F32)
    e16_d = din("e16", [8, 128], F32)
    bv_d = din("bv", [C], F32)
    out_d = nc.declare_dram_parameter("out", [128, CT * Q], BF16, isOutput=True)

    from contextlib import ExitStack
    with tile.TileContext(nc) as tc, ExitStack() as ctx:
        cst = ctx.enter_context(tc.tile_pool(name="cst", bufs=1))
        big = ctx.enter_context(tc.tile_pool(name="big", bufs=1))
        wrk = ctx.enter_context(tc.tile_pool(name="wrk", bufs=3))
        epool = ctx.enter_context(tc.tile_pool(name="epool", bufs=4))
        ps_s = ctx.enter_context(tc.tile_pool(name="ps_s", bufs=2, space="PSUM"))
        ps_o = ctx.enter_context(tc.tile_pool(name="ps_o", bufs=1, space="PSUM"))
        ps_mm = ctx.enter_context(tc.tile_pool(name="ps_mm", bufs=2, space="PSUM"))

        wtin = wrk.tile([1, 4], F32, name="wtin", bufs=1)
        nc.vector.memset(wtin[:], 0.0)
        wtout = wrk.tile([1, 4], F32, name="wtout", bufs=1)
        wrk_early = (wtin, wtout)

        # ---- input DMA: few big linear descriptors over 5 queues ----
        cpk = cst.tile([128, 36], F32)
        nc.scalar.dma_start(cpk[:], cpack_d[:])
        e16 = cst.tile([8, 128], F32)
        nc.scalar.dma_start(e16[:], e16_d[:])
        bq_c, bk_c, bp_c = cpk[:, 0:4], cpk[:, 4:8], cpk[:, 8:12]
        gqs_c, gqb_c = cpk[:, 12:16], cpk[:, 16:20]
        gks_c, gkb_c = cpk[:, 20:24], cpk[:, 24:28]
        g16 = cpk[:, 28:36]

        kvt = []
        for t in range(CT):
            kt_ = big.tile([128, HWF], BF16, name=f"kv{t}")
            nc.gpsimd.dma_start(kt_[:], kv_d[:, t * HWF : (t + 1) * HWF])
            kvt.append(kt_)
        xs = big.tile([128, CT * Q], BF16, name="xs")
        nc.sync.dma_start(xs[:], xs_d[:])
        xo = big.tile([128, CT * Q], BF16, name="xo")
        nc.gpsimd.dma_start(xo[:], xo_d[:])
        # warmup: preload the exp/ln ACT table set before weight postings
        wtin = wrk_early[0]
        wtout = wrk_early[1]
        nc.scalar.activation(wtout[:], wtin[:],
                             mybir.ActivationFunctionType.Exp)
        wk_sb = big.tile([128, CT * C], BF16, name="wk")
        nc.scalar.dma_start(wk_sb[:], wk_d[:])
        wq_sb = big.tile([128, CT * C], BF16, name="wq")
        nc.scalar.dma_start(wq_sb[:], wq_d[:])
        wv_sb = big.tile([128, CT * C], BF16, name="wv")
        nc.sync.dma_start(wv_sb[:], wv_d[:])
        wp_sb = big.tile([128, CT * C], BF16, name="wp")
        nc.sync.dma_start(wp_sb[:], wp_d[:])
        bv_ap = bv_d[:]
        bvbc = cst.tile([128, C], F32)
        nc.gpsimd.dma_start(
            out=bvbc[:],
            in_=bass.AP(tensor=bv_ap.tensor, offset=bv_ap.offset,
                        ap=[[0, 128]] + list(bv_ap.ap)),
        )

        # ---- groupnorm affine coefficients (a, b per channel) ----
        def gn_coeffs(src_chunks, gam, bet, label):
            statc = wrk.tile([128, 8], F32, name=f"statc_{label}", bufs=1)
            for t in range(CT):
                chunks = src_chunks[t]
                bnst = wrk.tile([128, len(chunks), 6], F32,
                                name=f"bnst_{label}", tag="bnst")
                for half, chunk in enumerate(chunks):
                    nc.vector.bn_stats(out=bnst[:, half, :], in_=chunk)
                mv = wrk.tile([128, 2], F32, name=f"mv_{label}", tag="mv")
                nc.vector.bn_aggr(out=mv[:], in_=bnst[:])
                nc.vector.tensor_copy(statc[:, t : t + 1], mv[:, 0:1])
                msq = wrk.tile([128, 1], F32, name=f"msq_{label}", tag="msq")
                nc.vector.tensor_mul(msq[:], mv[:, 0:1], mv[:, 0:1])
                nc.vector.tensor_add(statc[:, 4 + t : 5 + t], msq[:], mv[:, 1:2])
            gps = ps_mm.tile([128, 512], F32, name=f"gps_{label}", tag="mm")
            nc.tensor.matmul(gps[0:8, 0:8], lhsT=g16, rhs=statc[:],
                             start=True, stop=True)
            gs = wrk.tile([8, 8], F32, name=f"gs_{label}", tag="gs")
            nc.vector.tensor_copy(gs[:], gps[0:8, 0:8])
            ms = wrk.tile([8, 8], F32, name=f"ms_{label}", tag="ms")
            nc.vector.tensor_scalar_mul(ms[:], gs[:], 1.0 / GPC)
            msq8 = wrk.tile([8, 4], F32, name=f"msq8_{label}", tag="msq8")
            nc.vector.tensor_mul(msq8[:], ms[:, 0:4], ms[:, 0:4])
            var8 = wrk.tile([8, 4], F32, name=f"var8_{label}", tag="var8")
            nc.vector.tensor_sub(var8[:], ms[:, 4:8], msq8[:])
            # rstd = exp(-0.5*ln(var+eps)) — Ln/Exp share one ACT table set
            lnv = wrk.tile([8, 4], F32, name=f"lnv_{label}", tag="lnv")
            eps8 = wrk.tile([8, 1], F32, name=f"eps8_{label}", tag="eps8")
            nc.vector.memset(eps8[:], EPS)
            nc.scalar.activation(lnv[:], var8[:],
                                 mybir.ActivationFunctionType.Ln, bias=eps8[:])
            rhs2 = wrk.tile([8, 8], F32, name=f"rhs2_{label}", tag="rhs2", bufs=1)
            nc.scalar.activation(rhs2[:, 0:4], lnv[:],
                                 mybir.ActivationFunctionType.Exp, scale=-0.5)
            nc.vector.tensor_copy(rhs2[:, 4:8], ms[:, 0:4])
            pcs = ps_mm.tile([128, 512], F32, name=f"pcs_{label}", tag="mm")
            nc.tensor.matmul(pcs[:, 0:8], lhsT=e16[:], rhs=rhs2[:],
                             start=True, stop=True)
            pc = wrk.tile([128, 8], F32, name=f"pc_{label}", tag="pc")
            nc.vector.tensor_copy(pc[:], pcs[:, 0:8])
            a = wrk.tile([128, 4], F32, name=f"a_{label}", bufs=1)
            nc.vector.tensor_mul(a[:], pc[:, 0:4], gam)
            tmpb = wrk.tile([128, 4], F32, name=f"tmpb_{label}", tag="tmpb")
            nc.vector.tensor_mul(tmpb[:], pc[:, 4:8], a[:])
            b = wrk.tile([128, 4], F32, name=f"b_{label}", bufs=1)
            nc.vector.tensor_sub(b[:], bet, tmpb[:])
            return a, b

        akv, bkv = gn_coeffs(
            [(kvt[t][:, 0:512], kvt[t][:, 512:1024]) for t in range(CT)],
            gks_c, gkb_c, "kv")

        kvn = []
        for t in range(CT):
            kh = big.tile([128, HWF], DT, name=f"kvn{t}")
            nc.vector.tensor_scalar(
                out=kh[:], in0=kvt[t][:],
                scalar1=akv[:, t : t + 1], scalar2=bkv[:, t : t + 1],
                op0=mybir.AluOpType.mult, op1=mybir.AluOpType.add)
            kvn.append(kh)

        ax, bx = gn_coeffs(
            [(xs[:, t * Q : t * Q + 512], xo[:, t * Q : t * Q + 512])
             for t in range(CT)],
            gqs_c, gqb_c, "x")
        qin = []
        for t in range(CT):
            qt = big.tile([128, Q], DT, name=f"qin{t}")
            nc.vector.tensor_scalar(
                out=qt[:], in0=xs[:, t * Q : (t + 1) * Q],
                scalar1=ax[:, t : t + 1], scalar2=bx[:, t : t + 1],
                op0=mybir.AluOpType.mult, op1=mybir.AluOpType.add)
            qin.append(qt)

        k_sb = [None] * CT
        q_sb = [None] * CT
        vT_sb = [None] * KT

        def emit_k(m):
            kt_ = big.tile([128, HWF], DT, name=f"k{m}")
            for nh in range(2):
                ps = ps_mm.tile([128, 512], F32, name=f"psk{m}{nh}", tag="mm")
                for k in range(CT):
                    nc.tensor.matmul(
                        ps[:],
                        lhsT=wk_sb[:, k * C + m * 128 : k * C + (m + 1) * 128],
                        rhs=kvn[k][:, nh * 512 : (nh + 1) * 512],
                        start=(k == 0), stop=(k == CT - 1))
                nc.vector.tensor_scalar_add(
                    kt_[:, nh * 512 : (nh + 1) * 512], ps[:],
                    bk_c[:, m : m + 1])
            k_sb[m] = kt_

        def emit_q(m):
            ps = ps_mm.tile([128, 512], F32, name=f"psq{m}", tag="mm")
            for k in range(CT):
                nc.tensor.matmul(
                    ps[:],
                    lhsT=wq_sb[:, k * C + m * 128 : k * C + (m + 1) * 128],
                    rhs=qin[k][:], start=(k == 0), stop=(k == CT - 1))
            qt = big.tile([128, Q], DT, name=f"q{m}")
            nc.vector.tensor_scalar_add(qt[:], ps[:], bq_c[:, m : m + 1])
            q_sb[m] = qt

        def emit_v(mt):
            vt = big.tile([128, NH * (CHD + 1)], DT, name=f"vT{mt}")
            ones_col = vt[:].rearrange("p (h c) -> p h c", c=CHD + 1)[
                :, :, CHD : CHD + 1]
            nc.vector.memset(ones_col, 1.0)
            ps = ps_mm.tile([128, 512], F32, name=f"psv{mt}", tag="mm")
            for k in range(CT):
                nc.tensor.matmul(
                    ps[:], lhsT=kvn[k][:, mt * 128 : (mt + 1) * 128],
                    rhs=wv_sb[:, k * C : (k + 1) * C],
                    start=(k == 0), stop=(k == CT - 1))
            nc.vector.tensor_tensor(
                out=vt[:].rearrange("p (h c) -> p h c", c=CHD + 1)[:, :, 0:CHD],
                in0=ps[:].rearrange("p (h c) -> p h c", c=CHD),
                in1=bvbc[:].rearrange("p (h c) -> p h c", c=CHD),
                op=mybir.AluOpType.add)
            vT_sb[mt] = vt

        # ---- attention machinery (head pairs t) ----
        # zexp row-64 ones for the two K=1 broadcast expanders per pair
        zexp = cst.tile([65, 128], DT, name="zexp")
        nc.vector.memset(zexp[64:65, :], 1.0)

        et_tiles = {}

        def emit_scores(t, mk):
            pss = ps_s.tile([128, 1024], F32, name=f"pss{t}{mk}", tag="s")
            nc.tensor.matmul(pss[:, 0:512],
                             lhsT=k_sb[t][0:64, mk * 128 : (mk + 1) * 128],
                             rhs=q_sb[t][0:64, :],
                             start=True, stop=True, tile_position=(0, 0))
            nc.tensor.matmul(pss[:, 512:1024],
                             lhsT=k_sb[t][64:128, mk * 128 : (mk + 1) * 128],
                             rhs=q_sb[t][64:128, :],
                             start=True, stop=True, tile_position=(64, 0))
            et = epool.tile([128, 1024], DT, name=f"e{t}{mk}", tag="e")
            nc.scalar.activation(et[:], pss[:],
                                 mybir.ActivationFunctionType.Exp,
                                 scale=float(CHD) ** -0.5)
            et_tiles[(t, mk)] = et

        po_pair = {}

        def emit_av(t, mk):
            if mk == 0:
                po_pair[t] = (
                    ps_o.tile([65, 512], F32, name=f"poA{t}", tag="oA"),
                    ps_o.tile([65, 512], F32, name=f"poB{t}", tag="oB"),
                )
            poA, poB = po_pair[t]
            et = et_tiles[(t, mk)]
            nc.tensor.matmul(poA[:],
                             lhsT=vT_sb[mk][:, 130 * t : 130 * t + 65],
                             rhs=et[:, 0:512],
                             start=(mk == 0), stop=(mk == KT - 1))
            nc.tensor.matmul(poB[:],
                             lhsT=vT_sb[mk][:, 130 * t + 65 : 130 * t + 130],
                             rhs=et[:, 512:1024],
                             start=(mk == 0), stop=(mk == KT - 1))

        on_sb = [None] * CT

        def finish_pair(t):
            """AV(t,7) done: stage outputs, 1/Z, broadcast, normalize."""
            poA, poB = po_pair[t]
            ost = wrk.tile([128, 512], F32, name=f"ost{t}", tag="ost", bufs=2)
            nc.vector.tensor_copy(ost[0:64, :], poA[0:64, :])
            stB = wrk.tile([65, 512], F32, name=f"stB{t}", tag="stB", bufs=2)
            nc.vector.tensor_copy(stB[0:65, :], poB[0:65, :])
            # 1/Z = exp(-ln Z) off the PSUM Z-rows (row 64 = ones-col matmul);
            # Ln/Exp share the softmax ACT table set
            lnz = wrk.tile([65, 1024], F32, name=f"lnz{t}", tag="lnz", bufs=2)
            nc.scalar.activation(lnz[64:65, 0:512], poA[64:65, :],
                                 mybir.ActivationFunctionType.Ln)
            nc.scalar.activation(lnz[64:65, 512:1024], poB[64:65, :],
                                 mybir.ActivationFunctionType.Ln)
            rzb = wrk.tile([65, 1024], DT, name=f"rzb{t}", tag="rzb", bufs=2)
            nc.scalar.activation(rzb[64:65, :], lnz[64:65, :],
                                 mybir.ActivationFunctionType.Exp, scale=-1.0)
            # B-half data under A (partition shift via DMA)
            nc.sync.dma_start(ost[64:128, :], stB[0:64, :])
            # zps[0:64] = 1/Z_A broadcast, zps[64:128] = 1/Z_B broadcast
            zps = ps_mm.tile([128, 512], F32, name=f"zps{t}", tag="mm")
            nc.tensor.matmul(zps[0:64, :], lhsT=zexp[64:65, 0:64],
                             rhs=rzb[64:65, 0:512], start=True, stop=True,
                             tile_position=(64, 0))
            nc.tensor.matmul(zps[64:128, :], lhsT=zexp[64:65, 64:128],
                             rhs=rzb[64:65, 512:1024], start=True, stop=True,
                             tile_position=(64, 64))
            ont = big.tile([128, Q], DT, name=f"on{t}")
            nc.vector.tensor_mul(ont[:], ost[:], zps[:])
            on_sb[t] = ont

        # proj: SBUF f32 accumulation so PSUM stays free for attention
        acc = [big.tile([128, Q], F32, name=f"acc{m}") for m in range(CT)]
        xbp = [big.tile([128, Q], F32, name=f"xbp{m}") for m in range(CT)]

        def emit_xbp(m):
            nc.vector.tensor_scalar_add(
                xbp[m][:], xs[:, m * Q : (m + 1) * Q], bp_c[:, m : m + 1])

        def emit_proj(m, k):
            ps = ps_mm.tile([128, 512], F32, name=f"psp{m}{k}", tag="mm")
            nc.tensor.matmul(
                ps[:], lhsT=wp_sb[:, k * C + m * 128 : k * C + (m + 1) * 128],
                rhs=on_sb[k][:], start=True, stop=True)
            if k == 0:
                nc.vector.tensor_tensor(out=acc[m][:], in0=ps[:], in1=xbp[m][:],
                                        op=mybir.AluOpType.add)
            elif k < CT - 1:
                nc.vector.tensor_tensor(out=acc[m][:], in0=ps[:], in1=acc[m][:],
                                        op=mybir.AluOpType.add)
            else:
                r2 = wrk.tile([128, Q], BF16, name=f"r2_{m}", tag="r2", bufs=2)
                nc.vector.tensor_tensor(out=r2[:], in0=ps[:], in1=acc[m][:],
                                        op=mybir.AluOpType.add)
                eng = nc.sync if m % 2 == 0 else nc.gpsimd
                eng.dma_start(out_d[:, m * Q : (m + 1) * Q], r2[:])

        # ------------------ tensor-engine program order ------------------
        emit_k(0)
        emit_q(0)
        for m in range(CT):
            emit_xbp(m)
        emit_scores(0, 0)
        emit_scores(0, 1)
        emit_v(0)
        emit_v(1)
        # t=0: AV paced by exp; v tiles + k1/q1 as filler
        for mk in range(KT):
            emit_av(0, mk)
            if mk + 2 < KT:
                emit_scores(0, mk + 2)
            if mk + 2 < KT:
                emit_v(mk + 2)
        emit_k(1)
        emit_q(1)
        finish_pair(0)
        emit_scores(1, 0)
        emit_scores(1, 1)
        # t=1..3 with filler: k2/q2 during t=1, k3/q3 during t=2,
        # proj rounds as on(t) becomes available during t>=1
        filler = {1: [lambda: emit_k(2), lambda: emit_q(2),
                      lambda: emit_proj(0, 0), lambda: emit_proj(1, 0),
                      lambda: emit_proj(2, 0), lambda: emit_proj(3, 0)],
                  2: [lambda: emit_k(3), lambda: emit_q(3),
                      lambda: emit_proj(0, 1), lambda: emit_proj(1, 1),
                      lambda: emit_proj(2, 1), lambda: emit_proj(3, 1)],
                  3: [lambda: emit_proj(0, 2), lambda: emit_proj(1, 2),
                      lambda: emit_proj(2, 2), lambda: emit_proj(3, 2)]}
        for t in range(1, CT):
            fl = list(filler[t])
            for mk in range(KT):
                emit_av(t, mk)
                if mk + 2 < KT:
                    emit_scores(t, mk + 2)
                if fl:
                    fl.pop(0)()
            for f in fl:
                f()
            finish_pair(t)
            if t + 1 < CT:
                emit_scores(t + 1, 0)
                emit_scores(t + 1, 1)
        for m in range(CT):
            emit_proj(m, 3)

    _split_multi_waits(nc)
    return nc


_NC_CACHE = None
LAST_EXEC_NS = None


def kernel(**inputs):
    global _NC_CACHE, LAST_EXEC_NS
    import ml_dtypes
    bf = ml_dtypes.bfloat16

    x = np.asarray(inputs["x"], dtype=np.float32)
    kv = np.asarray(inputs["kv"], dtype=np.float32)

    def wtile(w):
        # [out,in] -> lhsT tiles: [128 p=in-chan within k-tile, k*512 + out]
        wT = np.asarray(w, np.float32).T  # [in, out]
        t = wT.reshape(CT, 128, C).transpose(1, 0, 2).reshape(128, CT * C)
        return np.ascontiguousarray(t).astype(bf)

    wkT = wtile(inputs["wk"])
    wqT = wtile(inputs["wq"])
    wvT = wtile(inputs["wv"])
    wpT = wtile(inputs["wproj"])
    bq = np.asarray(inputs["bq"], np.float32)
    bk = np.asarray(inputs["bk"], np.float32)
    bv = np.asarray(inputs["bv"], np.float32)
    bp = np.asarray(inputs["bproj"], np.float32)
    gqs = np.asarray(inputs["gnq_scale"], np.float32)
    gqb = np.asarray(inputs["gnq_bias"], np.float32)
    gks = np.asarray(inputs["gnkv_scale"], np.float32)
    gkb = np.asarray(inputs["gnkv_bias"], np.float32)

    p = np.arange(128)
    g16 = (p[:, None] // GPC == np.arange(8)[None, :]).astype(np.float32)
    e16 = np.ascontiguousarray(g16.T)
    cpack = np.concatenate(
        [v.reshape(4, 128).T for v in (bq, bk, bp, gqs, gqb, gks, gkb)]
        + [g16], axis=1).astype(np.float32)
    cpack = np.ascontiguousarray(cpack)

    def ctile(a):  # [C, n] -> [128, CT*n] tile-major
        n = a.shape[1]
        return np.ascontiguousarray(
            a.reshape(CT, 128, n).transpose(1, 0, 2).reshape(128, CT * n))

    xr = x.reshape(B, C, HWF)
    kvr = kv.reshape(B, C, HWF)

    in_maps = []
    for core in range(8):
        b, s = core // 2, core % 2
        in_maps.append({
            "kv": ctile(kvr[b]).astype(bf),
            "xs": ctile(xr[b][:, s * Q : (s + 1) * Q]).astype(bf),
            "xo": ctile(xr[b][:, (1 - s) * Q : (2 - s) * Q]).astype(bf),
            "wk": wkT, "wq": wqT, "wv": wvT, "wp": wpT,
            "bv": bv, "cpack": cpack, "e16": e16,
        })

    if _NC_CACHE is None:
        _NC_CACHE = build_program()

    trace = os.environ.get("BASS_ATTN_TRACE", "0") == "1"
    res = run_bass_kernel_spmd(_NC_CACHE, in_maps, core_ids=list(range(8)),
                               trace=trace)
    LAST_EXEC_NS = res.exec_time_ns

    out = np.empty((B, C, HWF), np.float32)
    for core in range(8):
        b, s = core // 2, core % 2
        o = np.asarray(res.results[core]["out"]).astype(np.float32)
        o = o.reshape(128, CT, Q).transpose(1, 0, 2).reshape(C, Q)
        out[b][:, s * Q : (s + 1) * Q] = o
    return out.reshape(B, C, H, W)
